# revision 5
# baseline (speedup 1.0000x reference)
"""Trainium2 Bass kernel for nn_DNM_Conv (LayerNorm -> synapse contraction ->
dendritic weighting -> GELU -> residual multiply).

Algebraic reduction of the reference:
    y = LayerNorm(x)                                  (b, n, d)
    t[b,o,d] = sum_n W[o,n] * y[b,n,d] + c[o]
        where W[o,n] = sum_m dw[o,m]*sw[o,m,n],  c[o] = sum_{m,n} dw[o,m]*sb[o,m,n]
    out = x * (gelu_erf(t) + 1)                       (o == n == 196)

All of LayerNorm is folded into the matmul on the host (stats over x are a
single cheap numpy pass; the 99%-of-FLOPs contraction stays on device):
    t[b] = Wr_ext[b].T @ x_ext[b]
  with x_ext[b] = [x[b]*ln_w ; ln_w ; ln_b ; ones]      (199, 768) rows
       Wr_ext[b] = [W.T*rstd[b] ; -B[b] ; S[b] ; c]     (199, 196) rows
       B[b,o] = sum_n W[o,n]*rstd[b,n]*mu[b,n],  S[b,o] = sum_n W[o,n]*rstd[b,n]
so pm = sum_n W*rstd*(x*ln_w) + ln_w*(-B) + ln_b*S + c = t exactly, and the
GELU needs no per-batch bias operand -> one wide ACT instruction per
(pair, o-chunk).  Device pipeline per (batch-pair, o-chunk):
    PE: 4 LdW + 8 matmuls -> PSUM [on, 1536]
    ACT: gelu [on, 1536] -> SBUF fp16
    DVE: (g+1) tensor_scalar, then *x tensor_tensor
    gpsimd queue: plain store DMA (no residual seed / accumulate RMW)

Distribution: data-parallel over batch, 8 batches per core on 8 cores.
Datapath fp16 (host casts), fp32 PSUM accumulation.
"""

import numpy as np

B, N, D, O, M = 64, 196, 768, 196, 2
N_CORES = 8
BPC = B // N_CORES          # batches per core
NPAIR = BPC // 2            # batch pairs (DMA + PSUM granularity)
NEXT = N + 3                # x rows + [ln_w, ln_b, ones] bias rows
NA, NB = 128, NEXT - 128    # contraction (n) partition split: 128 + 71
OA, OB = 128, O - 128       # output (o) partition split: 128 + 68
LN_EPS = 1e-5

_NC_CACHE = {}


def _build_nc(raw_final=False):
    """raw_final: load a separate raw-x tensor for the residual multiply
    (needed when ln_w != 1, since the matmul rhs is pre-scaled by ln_w)."""
    import concourse.bacc as bacc
    import concourse.tile as tile
    from concourse import mybir
    from contextlib import ExitStack

    F32 = mybir.dt.float32
    F16 = mybir.dt.float16
    AF = mybir.ActivationFunctionType
    OP = mybir.AluOpType

    nc = bacc.Bacc()
    x_d = nc.declare_dram_parameter("x", [BPC, NEXT, D], F16, isOutput=False)
    w_d = nc.declare_dram_parameter("w", [BPC, NEXT, O], F16, isOutput=False)
    if raw_final:
        xr_d = nc.declare_dram_parameter("xr", [BPC, N, D], F16, isOutput=False)
    out_d = nc.declare_dram_parameter("out", [BPC, N, D], F16, isOutput=True)

    x_pair = x_d.ap().rearrange("(q j) n d -> q n j d", j=2)    # (4, 199, 2, 768)
    out_pair = out_d.ap().rearrange("(q j) n d -> q n j d", j=2)
    if raw_final:
        xr_pair = xr_d.ap().rearrange("(q j) n d -> q n j d", j=2)

    nsplit = ((0, NA), (NA, NB))
    osplit = ((0, OA), (OA, OB))

    with tile.TileContext(nc) as tc, ExitStack() as ctx:
        xpool = ctx.enter_context(tc.tile_pool(name="xpool", bufs=NPAIR))
        wpool = ctx.enter_context(tc.tile_pool(name="wpool", bufs=BPC))
        gpool = ctx.enter_context(tc.tile_pool(name="gpool", bufs=2))
        opool = ctx.enter_context(tc.tile_pool(name="opool", bufs=3))
        psum = ctx.enter_context(tc.tile_pool(name="psum", bufs=1, space="PSUM"))

        # ---- all loads upfront (sync queue; transfers overlap compute) ----
        xtiles = []   # [q][ci] -> (pn, 2, 768) fp16
        rtiles = []   # [q][oc] -> raw-x tiles for the residual multiply
        wtiles = []   # [b][ci] -> (pn, 196) fp16
        for q in range(NPAIR):
            xq = []
            for ci, (p0, pn) in enumerate(nsplit):
                xt = xpool.tile([pn, 2, D], F16, tag=f"x{ci}", name=f"x{q}_{ci}")
                nc.sync.dma_start(out=xt[:], in_=x_pair[q, p0:p0 + pn, :, :])
                xq.append(xt)
            xtiles.append(xq)
            if raw_final:
                rq = []
                for oc, (o0, on) in enumerate(osplit):
                    rt = xpool.tile([on, 2, D], F16, tag=f"xr{oc}",
                                    name=f"xr{q}_{oc}")
                    nc.sync.dma_start(out=rt[:], in_=xr_pair[q, o0:o0 + on, :, :])
                    rq.append(rt)
                rtiles.append(rq)
            for j in range(2):
                b = 2 * q + j
                wb = []
                for ci, (p0, pn) in enumerate(nsplit):
                    wt = wpool.tile([pn, O], F16, tag=f"w{ci}", name=f"w{b}_{ci}")
                    nc.sync.dma_start(out=wt[:], in_=w_d[b, p0:p0 + pn, :])
                    wb.append(wt)
                wtiles.append(wb)

        # ---- pipeline over (pair, o-chunk) units ----
        for q in range(NPAIR):
            for oc, (o0, on) in enumerate(osplit):
                pm = psum.tile([on, 2, D], F32, tag=f"pm{oc}")
                for j in range(2):
                    b = 2 * q + j
                    # split on the absolute PSUM 512-col bank grid
                    csplit = ((0, 512), (512, 256)) if j == 0 else \
                             ((0, 256), (256, 512))
                    for ci, (p0, pn) in enumerate(nsplit):
                        w_sl = wtiles[b][ci][:, o0:o0 + on]
                        for c0, cn in csplit:
                            nc.tensor.matmul(
                                pm[:, j, c0:c0 + cn], w_sl,
                                xtiles[q][ci][:, j, c0:c0 + cn],
                                start=(ci == 0), stop=(ci == 1))
                g = gpool.tile([on, 2, D], F16, tag=f"g{oc}", name=f"g{q}_{oc}")
                nc.scalar.activation(out=g[:], in_=pm[:], func=AF.Gelu,
                                     bias=0.0, scale=1.0)
                # (g + 1) * x  -- o-chunk rows coincide with n-chunk rows
                nc.vector.tensor_scalar(out=g[:], in0=g[:], scalar1=1.0,
                                        scalar2=None, op0=OP.add)
                xres = (rtiles[q][oc][:] if raw_final
                        else xtiles[q][oc][0:on, :, :])
                ot = opool.tile([on, 2, D], F16, tag=f"o{oc}", name=f"o{q}_{oc}")
                nc.vector.tensor_tensor(out=ot[:], in0=g[:], in1=xres,
                                        op=OP.mult)
                nc.gpsimd.dma_start(out=out_pair[q, o0:o0 + on, :, :],
                                    in_=ot[:])

    nc.compile()
    return nc


def kernel(x, ln_w, ln_b, sw, sb, dw, _trace=False):
    from concourse.bass_utils import run_bass_kernel_spmd

    x = np.asarray(x, dtype=np.float32)
    ln_w = np.asarray(ln_w, dtype=np.float32)
    ln_b = np.asarray(ln_b, dtype=np.float32)
    sw = np.asarray(sw, dtype=np.float32)
    sb = np.asarray(sb, dtype=np.float32)
    dw = np.asarray(dw, dtype=np.float32)

    # Fold dendritic weights into the synapse contraction.
    W = np.einsum("om,omn->on", dw, sw)                     # (o, n)
    c = np.einsum("om,om->o", dw, sb.sum(-1))               # (o,)

    # LayerNorm statistics (one cheap fp32 pass; ~1% of total FLOPs).
    mu = x.mean(-1)                                         # (b, n)
    var = x.var(-1)
    rstd = 1.0 / np.sqrt(var + LN_EPS)                      # (b, n)

    Bterm = (rstd * mu) @ W.T                               # (b, o)
    Srow = rstd @ W.T                                       # (b, o)

    # x_ext rows: x*ln_w, then [ln_w, ln_b, ones]
    xs = np.empty((B, NEXT, D), dtype=np.float16)
    xs[:, 0:N, :] = (x * ln_w[None, None, :]).astype(np.float16)
    xs[:, N, :] = ln_w.astype(np.float16)
    xs[:, N + 1, :] = ln_b.astype(np.float16)
    xs[:, N + 2, :] = 1.0

    # Wr_ext rows: W.T * rstd, then [-B, S, c]
    WT = np.ascontiguousarray(W.T)                          # (n, o)
    wr = np.empty((B, NEXT, O), dtype=np.float16)
    wr[:, 0:N, :] = (WT[None, :, :] * rstd[:, :, None]).astype(np.float16)
    wr[:, N, :] = (-Bterm).astype(np.float16)
    wr[:, N + 1, :] = Srow.astype(np.float16)
    wr[:, N + 2, :] = c.astype(np.float16)

    trivial_ln = bool(np.all(ln_w == 1.0) and np.all(ln_b == 0.0))
    key = bool(trivial_ln)
    if key not in _NC_CACHE:
        _NC_CACHE[key] = _build_nc(raw_final=not trivial_ln)
    nc = _NC_CACHE[key]

    xraw16 = None if trivial_ln else x.astype(np.float16)
    in_maps = []
    for i in range(N_CORES):
        sl = slice(i * BPC, (i + 1) * BPC)
        m = {"x": xs[sl], "w": wr[sl]}
        if not trivial_ln:
            m["xr"] = xraw16[sl]
        in_maps.append(m)

    res = run_bass_kernel_spmd(nc, in_maps, core_ids=list(range(N_CORES)),
                               trace=_trace)
    out = np.concatenate([res.results[i]["out"] for i in range(N_CORES)],
                         axis=0).astype(np.float32)
    if _trace:
        return out, res
    return out


if __name__ == "__main__":
    pass


# revision 7
# speedup vs baseline: 1.1537x; 1.1537x over previous
"""Trainium2 Bass kernel for nn_DNM_Conv (LayerNorm -> synapse contraction ->
dendritic weighting -> GELU -> residual multiply).

Algebraic reduction of the reference:
    y = LayerNorm(x)                                  (b, n, d)
    t[b,o,d] = sum_n W[o,n] * y[b,n,d] + c[o]
        where W[o,n] = sum_m dw[o,m]*sw[o,m,n],  c[o] = sum_{m,n} dw[o,m]*sb[o,m,n]
    out = x * (gelu_erf(t) + 1)                       (o == n == 196)

All of LayerNorm is folded into the matmul on the host (stats over x are a
single cheap numpy pass; the 99%-of-FLOPs contraction stays on device):
    t[b] = Wr_ext[b].T @ x_ext[b]
  with x_ext[b] = [x[b]*ln_w ; ln_w ; ln_b ; ones]      (199, 768) rows
       Wr_ext[b] = [W.T*rstd[b] ; -B[b] ; S[b] ; c]     (199, 196) rows
       B[b,o] = sum_n W[o,n]*rstd[b,n]*mu[b,n],  S[b,o] = sum_n W[o,n]*rstd[b,n]
so pm = sum_n W*rstd*(x*ln_w) + ln_w*(-B) + ln_b*S + c = t exactly, and the
GELU needs no per-batch bias operand -> one wide ACT instruction per
(pair, o-chunk).  Device pipeline per (batch-pair, o-chunk):
    PE: 4 LdW + 8 matmuls -> PSUM [on, 1536]
    ACT: gelu [on, 1536] -> SBUF fp16
    DVE: (g+1) tensor_scalar, then *x tensor_tensor
    gpsimd queue: plain store DMA (no residual seed / accumulate RMW)

Distribution: data-parallel over batch, 8 batches per core on 8 cores.
Datapath fp16 (host casts), fp32 PSUM accumulation.
"""

import numpy as np

B, N, D, O, M = 64, 196, 768, 196, 2
N_CORES = 8
BPC = B // N_CORES          # batches per core
NPAIR = BPC // 2            # batch pairs (DMA + PSUM granularity)
NEXT = N + 3                # x rows + [ln_w, ln_b, ones] bias rows
NA, NB = 128, NEXT - 128    # contraction (n) partition split: 128 + 71
OA, OB = 128, O - 128       # output (o) partition split: 128 + 68
LN_EPS = 1e-5

_NC_CACHE = {}


def _build_nc(raw_final=False):
    """raw_final: load a separate raw-x tensor for the residual multiply
    (needed when ln_w != 1, since the matmul rhs is pre-scaled by ln_w)."""
    import concourse.bacc as bacc
    import concourse.tile as tile
    from concourse import mybir
    from contextlib import ExitStack

    F32 = mybir.dt.float32
    F16 = mybir.dt.float16
    AF = mybir.ActivationFunctionType
    OP = mybir.AluOpType

    nc = bacc.Bacc()
    # host pre-permutes to pair-interleaved [q, n, j, d] so every DMA is a
    # dense 2D pattern with 3KB contiguous rows
    x_d = nc.declare_dram_parameter("x", [NPAIR, NEXT, 2, D], F16, isOutput=False)
    w_d = nc.declare_dram_parameter("w", [BPC, NEXT, O], F16, isOutput=False)
    if raw_final:
        xr_d = nc.declare_dram_parameter("xr", [NPAIR, N, 2, D], F16,
                                         isOutput=False)
    out_d = nc.declare_dram_parameter("out", [NPAIR, N, 2, D], F16, isOutput=True)

    x_pair = x_d.ap()
    out_pair = out_d.ap()
    if raw_final:
        xr_pair = xr_d.ap()

    nsplit = ((0, NA), (NA, NB))
    osplit = ((0, OA), (OA, OB))

    with tile.TileContext(nc) as tc, ExitStack() as ctx:
        xpool = ctx.enter_context(tc.tile_pool(name="xpool", bufs=NPAIR))
        wpool = ctx.enter_context(tc.tile_pool(name="wpool", bufs=BPC))
        gpool = ctx.enter_context(tc.tile_pool(name="gpool", bufs=2))
        opool = ctx.enter_context(tc.tile_pool(name="opool", bufs=3))
        psum = ctx.enter_context(tc.tile_pool(name="psum", bufs=1, space="PSUM"))

        # ---- all loads upfront (sync queue; transfers overlap compute) ----
        xtiles = []   # [q][ci] -> (pn, 2, 768) fp16
        rtiles = []   # [q][oc] -> raw-x tiles for the residual multiply
        wtiles = []   # [b][ci] -> (pn, 196) fp16
        for q in range(NPAIR):
            xq = []
            for ci, (p0, pn) in enumerate(nsplit):
                xt = xpool.tile([pn, 2, D], F16, tag=f"x{ci}", name=f"x{q}_{ci}")
                nc.sync.dma_start(out=xt[:], in_=x_pair[q, p0:p0 + pn, :, :])
                xq.append(xt)
            xtiles.append(xq)
            if raw_final:
                rq = []
                for oc, (o0, on) in enumerate(osplit):
                    rt = xpool.tile([on, 2, D], F16, tag=f"xr{oc}",
                                    name=f"xr{q}_{oc}")
                    nc.sync.dma_start(out=rt[:], in_=xr_pair[q, o0:o0 + on, :, :])
                    rq.append(rt)
                rtiles.append(rq)
            for j in range(2):
                b = 2 * q + j
                wb = []
                for ci, (p0, pn) in enumerate(nsplit):
                    wt = wpool.tile([pn, O], F16, tag=f"w{ci}", name=f"w{b}_{ci}")
                    nc.gpsimd.dma_start(out=wt[:], in_=w_d[b, p0:p0 + pn, :])
                    wb.append(wt)
                wtiles.append(wb)

        # ---- pipeline over (pair, o-chunk) units ----
        for q in range(NPAIR):
            for oc, (o0, on) in enumerate(osplit):
                pm = psum.tile([on, 2, D], F32, tag=f"pm{oc}")
                for j in range(2):
                    b = 2 * q + j
                    # split on the absolute PSUM 512-col bank grid
                    csplit = ((0, 512), (512, 256)) if j == 0 else \
                             ((0, 256), (256, 512))
                    for ci, (p0, pn) in enumerate(nsplit):
                        w_sl = wtiles[b][ci][:, o0:o0 + on]
                        for c0, cn in csplit:
                            nc.tensor.matmul(
                                pm[:, j, c0:c0 + cn], w_sl,
                                xtiles[q][ci][:, j, c0:c0 + cn],
                                start=(ci == 0), stop=(ci == 1))
                g = gpool.tile([on, 2, D], F16, tag=f"g{oc}", name=f"g{q}_{oc}")
                nc.scalar.activation(out=g[:], in_=pm[:], func=AF.Gelu,
                                     bias=0.0, scale=1.0)
                # (g + 1) * x  -- o-chunk rows coincide with n-chunk rows
                nc.vector.tensor_scalar(out=g[:], in0=g[:], scalar1=1.0,
                                        scalar2=None, op0=OP.add)
                xres = (rtiles[q][oc][:] if raw_final
                        else xtiles[q][oc][0:on, :, :])
                ot = opool.tile([on, 2, D], F16, tag=f"o{oc}", name=f"o{q}_{oc}")
                nc.vector.tensor_tensor(out=ot[:], in0=g[:], in1=xres,
                                        op=OP.mult)
                nc.gpsimd.dma_start(out=out_pair[q, o0:o0 + on, :, :],
                                    in_=ot[:])

    nc.compile()
    return nc


def kernel(x, ln_w, ln_b, sw, sb, dw, _trace=False):
    from concourse.bass_utils import run_bass_kernel_spmd

    x = np.asarray(x, dtype=np.float32)
    ln_w = np.asarray(ln_w, dtype=np.float32)
    ln_b = np.asarray(ln_b, dtype=np.float32)
    sw = np.asarray(sw, dtype=np.float32)
    sb = np.asarray(sb, dtype=np.float32)
    dw = np.asarray(dw, dtype=np.float32)

    # Fold dendritic weights into the synapse contraction.
    W = np.einsum("om,omn->on", dw, sw)                     # (o, n)
    c = np.einsum("om,om->o", dw, sb.sum(-1))               # (o,)

    # LayerNorm statistics (one cheap fp32 pass; ~1% of total FLOPs).
    mu = x.mean(-1)                                         # (b, n)
    var = x.var(-1)
    rstd = 1.0 / np.sqrt(var + LN_EPS)                      # (b, n)

    Bterm = (rstd * mu) @ W.T                               # (b, o)
    Srow = rstd @ W.T                                       # (b, o)

    # x_ext rows: x*ln_w, then [ln_w, ln_b, ones]
    xs = np.empty((B, NEXT, D), dtype=np.float16)
    xs[:, 0:N, :] = (x * ln_w[None, None, :]).astype(np.float16)
    xs[:, N, :] = ln_w.astype(np.float16)
    xs[:, N + 1, :] = ln_b.astype(np.float16)
    xs[:, N + 2, :] = 1.0

    # Wr_ext rows: W.T * rstd, then [-B, S, c]
    WT = np.ascontiguousarray(W.T)                          # (n, o)
    wr = np.empty((B, NEXT, O), dtype=np.float16)
    wr[:, 0:N, :] = (WT[None, :, :] * rstd[:, :, None]).astype(np.float16)
    wr[:, N, :] = (-Bterm).astype(np.float16)
    wr[:, N + 1, :] = Srow.astype(np.float16)
    wr[:, N + 2, :] = c.astype(np.float16)

    trivial_ln = bool(np.all(ln_w == 1.0) and np.all(ln_b == 0.0))
    key = bool(trivial_ln)
    if key not in _NC_CACHE:
        _NC_CACHE[key] = _build_nc(raw_final=not trivial_ln)
    nc = _NC_CACHE[key]

    xraw16 = None if trivial_ln else x.astype(np.float16)
    # pair-interleave: (b=2q+j, n, d) -> (q, n, j, d)
    xsp = np.ascontiguousarray(
        xs.reshape(N_CORES, NPAIR, 2, NEXT, D).transpose(0, 1, 3, 2, 4))
    if not trivial_ln:
        xrp = np.ascontiguousarray(
            xraw16.reshape(N_CORES, NPAIR, 2, N, D).transpose(0, 1, 3, 2, 4))
    in_maps = []
    for i in range(N_CORES):
        sl = slice(i * BPC, (i + 1) * BPC)
        m = {"x": xsp[i], "w": wr[sl]}
        if not trivial_ln:
            m["xr"] = xrp[i]
        in_maps.append(m)

    res = run_bass_kernel_spmd(nc, in_maps, core_ids=list(range(N_CORES)),
                               trace=_trace)
    out = np.stack([res.results[i]["out"] for i in range(N_CORES)])
    # (cores, q, n, j, d) -> (b, n, d)
    out = out.transpose(0, 1, 3, 2, 4).reshape(B, N, D).astype(np.float32)
    if _trace:
        return out, res
    return out


if __name__ == "__main__":
    pass
